# revision 47
# baseline (speedup 1.0000x reference)
"""VGCN encoder (2-layer GCN, shared normalized adjacency) on 8 Trainium2 cores.

Strategy: node-partitioned graph parallelism. Nodes are padded to
NPAD = 8*6272 and core c owns nodes [6272c, 6272(c+1)), split into 98 buckets
of 64. All edges (plus one self-edge per node, which realizes both GCN's +1
degree and the self-loop term) are routed to the core owning their dst node,
bucketed by dst bucket, and aggregated on-device with one-hot matmuls over
128-edge tiles (bf16 operands, fp32 psum):

    agg[bucket] += S.T @ us[src]   (S[e, j] = dst_local[e] == j, built on DVE)

Messages are fetched with SWDGE dma_gather (4 queues round-robin) from a
bf16 DRAM table whose 256-byte rows ([HID bf16 payload | HID pad], the
minimum SWDGE element) are stored in a (core, partition, bucket)-major
permutation so bulk table writes are contiguous DMAs; the host permutes
gather indices to match. dis = 1/sqrt(deg) is precomputed on the host (pure
graph structure, like the edge bucketing itself).

Layer 1 input (x @ W1, x shipped in bf16) is computed REPLICATED on every
core for the whole graph - it is ~50 MFLOP, far cheaper than an AllGather
round - so the only collective is the AllGather of the layer-1 activations
(packed bf16, Shared-scratchpad output, expanded to padded rows locally).
Layer-2 aggregation runs feature-major (lhsT=msg) so both output projections
become 13 wide matmuls against [Wmu | Wlv] with one transpose per 128 nodes.

All host-prepared operands travel in ONE packed int16 input tensor (f32/bf16
sections bitcast on device) and both outputs leave in ONE tensor; together
with pipelined dispatch this keeps the per-execution dispatch overhead of the
axon tunnel (~0.8 ms flat per round trip) mostly off the measured
steady-state time.
"""

import sys

sys.path.insert(0, "/opt/trn_rl_repo")

import numpy as np

from concourse import bacc, mybir, tile
from concourse.bass_utils import run_bass_kernel_spmd
from concourse.masks import make_identity

F32 = mybir.dt.float32
BF16 = mybir.dt.bfloat16
I16 = mybir.dt.int16
I32 = mybir.dt.int32


class Cfg:
    def __init__(self, n=50000, e=800000, in_dim=128, hid=64, ncores=8,
                 shard_tiles=49, bw=64, half=32768, chunk_tiles=38, sbatch=16):
        self.N, self.E, self.IN, self.HID = n, e, in_dim, hid
        self.NCORES = ncores
        self.P = 128
        self.SHARD = shard_tiles * 128    # nodes per core
        self.NPAD = ncores * self.SHARD
        self.BW = bw                      # bucket width (psum partition dim)
        self.NBK = self.SHARD // bw       # buckets per core
        self.GBK = ncores * self.NBK      # global buckets
        self.HALF = half                  # gather-table split so int16 idx fit
        self.CH = chunk_tiles             # tiles (128 rows) per dma_gather
        self.SB = sbatch                  # tiles per batched one-hot build
        assert self.NPAD >= n and half <= 32768 and self.SHARD % bw == 0
        assert self.NBK <= 128 and self.NBK % 2 == 0


DEFAULT = Cfg()


def pack_offsets(cfg, nTA, nTB):
    """Column offsets (int16 units) of each section in the packed input."""
    off, out = 0, {}
    for name, width in (("xT", cfg.NPAD), ("w1", 2 * cfg.HID),
                        ("wml", 4 * cfg.HID), ("dis", 2 * cfg.NBK),
                        ("disG", 2 * cfg.NCORES * cfg.NBK),
                        ("disP", cfg.NBK), ("dlA", 2 * nTA),
                        ("dlB", 2 * nTB), ("idxA", 8 * nTA),
                        ("idxB", 8 * nTB)):
        out[name] = off
        off += width
    out["W"] = off
    return out


def build_layout(edge_index, cfg=DEFAULT):
    """Static per-core edge streams plus the (identical-across-cores) tile
    structure. Table row of node n: c*SHARD + (r%BW)*NBK + r//BW, r=n%SHARD."""
    src = np.asarray(edge_index[0], np.int64)
    dst = np.asarray(edge_index[1], np.int64)
    NBK, BW = cfg.NBK, cfg.BW

    deg = np.bincount(dst, minlength=cfg.NPAD).astype(np.float64) + 1.0
    dis = (1.0 / np.sqrt(deg)).astype(np.float32)   # padding nodes: dis=1

    per_core = []
    cnts = np.zeros((cfg.NCORES, NBK * 2), np.int64)
    for c in range(cfg.NCORES):
        m = (dst >= c * cfg.SHARD) & (dst < (c + 1) * cfg.SHARD)
        s = src[m]
        d = dst[m]
        selfn = np.arange(c * cfg.SHARD, (c + 1) * cfg.SHARD, dtype=np.int64)
        s = np.concatenate([s, selfn])
        d = np.concatenate([d, selfn])
        cc, rr = s // cfg.SHARD, s % cfg.SHARD
        row = cc * cfg.SHARD + (rr % BW) * NBK + rr // BW
        dr = d - c * cfg.SHARD
        b = dr // BW
        dl = dr % BW
        h = (row >= cfg.HALF).astype(np.int64)
        key = b * 2 + h
        order = np.argsort(key, kind="stable")
        row, dl, key = row[order], dl[order], key[order]
        per_core.append((row, dl, key))
        cnts[c] = np.bincount(key, minlength=NBK * 2)

    ntile = np.ceil(cnts.max(axis=0) / 128.0).astype(np.int64)
    ntA, ntB = ntile[0::2], ntile[1::2]
    nTA, nTB = int(ntA.sum()), int(ntB.sum())

    tbA = np.repeat(np.arange(NBK), ntA)
    tbB = np.repeat(np.arange(NBK), ntB)
    offA = np.concatenate([[0], np.cumsum(ntA)]) * 128
    offB = np.concatenate([[0], np.cumsum(ntB)]) * 128

    cores = []
    for c in range(cfg.NCORES):
        row, dl, key = per_core[c]
        bounds = np.searchsorted(key, np.arange(NBK * 2 + 1))
        idxA = np.zeros(nTA * 128, np.int64)
        dlA = np.full(nTA * 128, BW, np.int64)
        idxB = np.zeros(nTB * 128, np.int64)
        dlB = np.full(nTB * 128, BW, np.int64)
        for b in range(NBK):
            lo, hi = bounds[2 * b], bounds[2 * b + 1]
            o = offA[b]
            idxA[o:o + hi - lo] = row[lo:hi]
            dlA[o:o + hi - lo] = dl[lo:hi]
            lo, hi = bounds[2 * b + 1], bounds[2 * b + 2]
            o = offB[b]
            idxB[o:o + hi - lo] = row[lo:hi] - cfg.HALF
            dlB[o:o + hi - lo] = dl[lo:hi]

        def wrap(stream):
            a = stream.reshape(-1, 16).T.astype(np.int16)
            return np.tile(a, (8, 1))   # replicated across the 8 q7 cores

        # dis for own nodes in (partition=dl, bucket) layout
        dis_own = np.ascontiguousarray(
            dis[c * cfg.SHARD:(c + 1) * cfg.SHARD].reshape(NBK, BW).T)

        disP = np.ascontiguousarray(
            dis_own.reshape(BW, NBK // 2, 2).transpose(2, 0, 1)
            .reshape(2 * BW, NBK // 2))

        cores.append(dict(
            idxA=wrap(idxA), idxB=wrap(idxB),
            dlA=np.ascontiguousarray(dlA.reshape(-1, 128).T.astype(np.float32)),
            dlB=np.ascontiguousarray(dlB.reshape(-1, 128).T.astype(np.float32)),
            dis=dis_own, disP=disP,
        ))

    disG = np.concatenate([c["dis"] for c in cores], axis=1)
    return dict(ntA=tuple(int(x) for x in ntA), ntB=tuple(int(x) for x in ntB),
                tbA=tbA, tbB=tbB, nTA=nTA, nTB=nTB, cores=cores, disG=disG)


def build_program(layout, cfg=DEFAULT, has_bias=False, reps=1,
                  skip_cc=False, phases=3, gather_mode="gather", proj=True):
    """Emit the SPMD bass program (identical on all cores)."""
    nc = bacc.Bacc("TRN2", target_bir_lowering=False, debug=False,
                   num_devices=cfg.NCORES, num_swdge_queues=4)
    P, BW, NBK, HID = cfg.P, cfg.BW, cfg.NBK, cfg.HID
    nTA, nTB = layout["nTA"], layout["nTB"]
    tb = {0: layout["tbA"], 1: layout["tbB"]}
    nT = {0: nTA, 1: nTB}
    HALVES = [H for H in (0, 1) if nT[H] > 0]
    use_cc = cfg.NCORES > 1 and not skip_cc

    # ---------------- I/O (single packed input, single output) ----------
    OFF = pack_offsets(cfg, nTA, nTB)
    pk_in = nc.dram_tensor("pk", [P, OFF["W"]], I16, kind="ExternalInput")

    def fsec(name, n, rows=P):
        o = OFF[name]
        return pk_in.ap()[:rows, o:o + 2 * n].bitcast(F32)

    def isec(name, n):
        o = OFF[name]
        return pk_in.ap()[:, o:o + n]

    xT_b = pk_in.ap()[:, OFF["xT"]:OFF["xT"] + cfg.NPAD].bitcast(BF16)
    if has_bias:
        b1_in = nc.dram_tensor("b1", [1, HID], F32, kind="ExternalInput")
        bmu_in = nc.dram_tensor("bmu", [1, HID], F32, kind="ExternalInput")
        blv_in = nc.dram_tensor("blv", [1, HID], F32, kind="ExternalInput")
    z_out = nc.dram_tensor("z", [P, NBK // 2, 2 * HID], BF16,
                           kind="ExternalOutput")

    with tile.TileContext(nc) as tc:
        import contextlib
        stack = contextlib.ExitStack()
        with stack:
            dram = stack.enter_context(tc.tile_pool(name="dram", bufs=1, space="DRAM"))
            cpool = stack.enter_context(tc.tile_pool(name="const", bufs=1))

            us_tab = dram.tile([cfg.NPAD, 2 * HID], BF16)
            hs2_bnc = dram.tile([cfg.SHARD, HID], BF16)
            hs2_pk = dram.tile([cfg.NPAD, HID], BF16, addr_space="Shared")
            hs2_tab = dram.tile([cfg.NPAD, 2 * HID], BF16)

            w1_f = cpool.tile([cfg.IN, HID], F32)
            nc.sync.dma_start(out=w1_f[:], in_=fsec("w1", HID))
            w1_sb = cpool.tile([cfg.IN, HID], BF16)
            nc.vector.tensor_copy(out=w1_sb[:], in_=w1_f[:])
            wml_f = cpool.tile([HID, 2 * HID], F32)
            nc.sync.dma_start(out=wml_f[:], in_=fsec("wml", 2 * HID, HID))
            wml_sb = cpool.tile([HID, 2 * HID], BF16)
            nc.vector.tensor_copy(out=wml_sb[:], in_=wml_f[:])
            dis_own = cpool.tile([BW, NBK], F32)
            nc.sync.dma_start(out=dis_own[:], in_=fsec("dis", NBK, BW))
            disP = cpool.tile([P, NBK // 2], F32)
            nc.sync.dma_start(out=disP[:], in_=fsec("disP", NBK // 2))
            dis2 = cpool.tile([BW, NBK], F32)
            nc.vector.tensor_tensor(out=dis2[:], in0=dis_own[:],
                                    in1=dis_own[:], op=mybir.AluOpType.mult)
            dis_g = cpool.tile([BW, cfg.NCORES * NBK], F32)
            nc.sync.dma_start(out=dis_g[:],
                              in_=fsec("disG", cfg.NCORES * NBK, BW))

            iota_i = cpool.tile([P, BW], I32)
            nc.gpsimd.iota(iota_i[:], pattern=[[1, BW]], base=0,
                           channel_multiplier=0)
            iota_b = cpool.tile([P, BW], BF16)
            nc.vector.tensor_copy(out=iota_b[:], in_=iota_i[:])

            ident = cpool.tile([P, P], F32)
            make_identity(nc, ident[:])
            ident_bf = cpool.tile([P, P], BF16)
            nc.vector.tensor_copy(out=ident_bf[:], in_=ident[:])

            idx_sb, dl_sb = {}, {}
            for H in HALVES:
                idx_sb[H] = cpool.tile([P, nT[H] * 8], I16, tag=f"idx{H}",
                                       name=f"idx{H}")
                nc.sync.dma_start(out=idx_sb[H][:],
                                  in_=isec("idxA" if H == 0 else "idxB",
                                           nT[H] * 8))
                dl_f = cpool.tile([P, nT[H]], F32, tag=f"dlf{H}",
                                  name=f"dlf{H}")
                nc.sync.dma_start(out=dl_f[:],
                                  in_=fsec("dlA" if H == 0 else "dlB",
                                           nT[H]))
                dl_sb[H] = cpool.tile([P, nT[H]], BF16, tag=f"dl{H}",
                                      name=f"dls{H}")
                nc.vector.tensor_copy(out=dl_sb[H][:], in_=dl_f[:])

            if has_bias:
                brow = cpool.tile([1, 3 * HID], F32)
                nc.sync.dma_start(out=brow[:, 0:HID], in_=b1_in.ap()[:])
                nc.sync.dma_start(out=brow[:, HID:2 * HID], in_=bmu_in.ap()[:])
                nc.sync.dma_start(out=brow[:, 2 * HID:], in_=blv_in.ap()[:])
                bias_bc = cpool.tile([P, 3 * HID], F32)
                nc.gpsimd.partition_broadcast(bias_bc[:], brow[:])

            def build_S(spool, H, tag):
                tiles = []
                for t0 in range(0, nT[H], cfg.SB):
                    tn = min(cfg.SB, nT[H] - t0)
                    st = spool.tile([P, cfg.SB, BW], BF16, tag=tag,
                                    name=f"S{tag}")
                    nc.vector.tensor_tensor(
                        out=st[:, :tn, :],
                        in0=dl_sb[H][:, t0:t0 + tn].to_broadcast([P, tn, BW]),
                        in1=iota_b[:, None, :].to_broadcast([P, tn, BW]),
                        op=mybir.AluOpType.is_equal,
                    )
                    tiles.append(st)

                def one(t):
                    return tiles[t // cfg.SB][:, t % cfg.SB, :]

                return one

            qctr = [0]

            def gather_all(mpool, table, tagp):
                """Gather both halves' chunks, interleaved so the earliest
                tiles of BOTH halves land first (bucket 0 needs both)."""
                tiles = {H: [] for H in HALVES}
                starts = {H: list(range(0, nT[H], cfg.CH)) for H in HALVES}
                order = []
                for i in range(max(len(starts[H]) for H in HALVES)):
                    for H in HALVES:
                        if i < len(starts[H]):
                            order.append((H, starts[H][i]))
                for H, t0 in order:
                    tn = min(cfg.CH, nT[H] - t0)
                    mt = mpool.tile([P, cfg.CH, 2 * HID], BF16,
                                    tag=f"{tagp}{H}", name=f"M{tagp}{H}")
                    if gather_mode == "copy":
                        nc.sync.dma_start(
                            out=mt[:, :tn, :],
                            in_=table[:tn * 128, :]
                            .rearrange("(t p) f -> p t f", p=128))
                    else:
                        nc.gpsimd.dma_gather(
                            out_ap=mt[:, :tn, :],
                            in_ap=(table[:min(cfg.HALF, cfg.NPAD), :]
                                   if H == 0 else table[cfg.HALF:, :]),
                            idxs_ap=idx_sb[H][:, t0 * 8:(t0 + tn) * 8],
                            num_idxs=tn * 128, num_idxs_reg=tn * 128,
                            elem_size=2 * HID,
                            single_packet=(tn * 128 <= 512),
                            queue_num=qctr[0] % 4,
                        )
                        qctr[0] += 1
                    tiles[H].append(mt)

                def make(H):
                    tl = tiles[H]
                    return lambda t: tl[t // cfg.CH][:, t % cfg.CH, 0:HID]

                return {H: make(H) for H in HALVES}

            entries = [[] for _ in range(NBK)]
            for H in HALVES:
                for t, b in enumerate(tb[H]):
                    entries[int(b)].append((H, t))

            for _rep in range(reps):
                # ========= PHASE A: u = x @ W1, scaled by dis -> us table ====
                it_stack = contextlib.ExitStack()
                with it_stack:
                    xa = it_stack.enter_context(tc.tile_pool(name="xa", bufs=2))
                    pu = it_stack.enter_context(
                        tc.tile_pool(name="pu", bufs=1, space="PSUM"))
                    usb = it_stack.enter_context(tc.tile_pool(name="usb", bufs=2))
                    spool = it_stack.enter_context(tc.tile_pool(name="spool", bufs=2))
                    mpool = it_stack.enter_context(tc.tile_pool(name="mpool", bufs=6))
                    pagg = it_stack.enter_context(
                        tc.tile_pool(name="pagg", bufs=3, space="PSUM"))
                    hb = it_stack.enter_context(tc.tile_pool(name="hb", bufs=2))
                    small = it_stack.enter_context(tc.tile_pool(name="small", bufs=3))
                    ptr = it_stack.enter_context(
                        tc.tile_pool(name="ptr", bufs=1, space="PSUM"))
                    pproj = it_stack.enter_context(
                        tc.tile_pool(name="pproj", bufs=2, space="PSUM"))
                    pz = it_stack.enter_context(
                        tc.tile_pool(name="pz", bufs=1, space="PSUM"))

                    XC = 16  # buckets per xT DMA / psum bank / scale batch
                    if phases < 1:
                        zfill0 = usb.tile([BW, NBK, HID], BF16, tag="usbb",
                                          name="zfill0")
                        nc.vector.memset(zfill0[:], 0)
                    c2_order = [5, 6, 7, 0, 1, 2, 3, 4][:cfg.NCORES]
                    for c2 in (c2_order if phases >= 1 else []):
                        us_blk = usb.tile([BW, NBK, HID], BF16, tag="usbb",
                                          name="us_blk")
                        for B0 in range(0, NBK, XC):
                            bn = min(XC, NBK - B0)
                            xtb = xa.tile([P, XC, BW], BF16, tag="xtb",
                                          name="xtb")
                            nc.sync.dma_start(
                                out=xtb[:, :bn, :],
                                in_=xT_b[:, c2 * cfg.SHARD + B0 * BW:
                                         c2 * cfg.SHARD + (B0 + bn) * BW]
                                .rearrange("p (t q) -> p t q", t=bn))
                            ups = pu.tile([BW, XC, HID], F32, space="PSUM",
                                          tag="u", name="ups")
                            for j in range(bn):
                                nc.tensor.matmul(out=ups[:, j, :],
                                                 lhsT=xtb[:, j, :],
                                                 rhs=w1_sb[:],
                                                 start=True, stop=True)
                            nc.vector.tensor_tensor(
                                out=us_blk[:, B0:B0 + bn, :],
                                in0=ups[:, :bn, :],
                                in1=dis_g[:, c2 * NBK + B0:
                                          c2 * NBK + B0 + bn, None]
                                .to_broadcast([BW, bn, HID]),
                                op=mybir.AluOpType.mult)
                        nc.sync.dma_start(
                            out=us_tab[c2 * cfg.SHARD:(c2 + 1) * cfg.SHARD,
                                       0:HID]
                            .rearrange("(j b) f -> j b f", j=BW),
                            in_=us_blk[:])

                    # ================= PHASE B: layer-1 aggregation =============
                    if phases >= 2:
                        msg = gather_all(mpool, us_tab[:], "m")
                        S1 = {H: build_S(spool, H, f"s{H}") for H in HALVES}
                        hs2_sb = usb.tile([BW, NBK, HID], BF16, tag="usbb",
                                          name="hs2_sb")
                        for b0 in range(0, NBK, 2):
                            ps = pagg.tile([BW, 2, HID], F32, space="PSUM",
                                           tag="agg", name="ps1")
                            for k in (0, 1):
                                ent = entries[b0 + k]
                                for i, (H, t) in enumerate(ent):
                                    nc.tensor.matmul(
                                        out=ps[:, k, :], lhsT=S1[H](t),
                                        rhs=msg[H](t), start=(i == 0),
                                        stop=(i == len(ent) - 1))
                            t1 = hb.tile([BW, 2, HID], F32, tag="h",
                                         name="t1")
                            if has_bias:
                                # h1 = relu(ps*dis + b); hs2 = h1*dis
                                dpair = dis_own[:, b0:b0 + 2, None] \
                                    .to_broadcast([BW, 2, HID])
                                nc.vector.tensor_tensor(
                                    out=t1[:], in0=ps[:], in1=dpair,
                                    op=mybir.AluOpType.mult)
                                nc.vector.tensor_tensor(
                                    out=t1[:], in0=t1[:],
                                    in1=bias_bc[:BW, None, 0:HID]
                                    .to_broadcast([BW, 2, HID]),
                                    op=mybir.AluOpType.add)
                                nc.vector.tensor_relu(out=t1[:], in_=t1[:])
                                nc.vector.tensor_tensor(
                                    out=hs2_sb[:, b0:b0 + 2, :], in0=t1[:],
                                    in1=dpair, op=mybir.AluOpType.mult)
                            else:
                                # dis>0: relu(ps*dis)*dis == relu(ps)*dis^2
                                nc.vector.tensor_relu(out=t1[:], in_=ps[:])
                                nc.vector.tensor_tensor(
                                    out=hs2_sb[:, b0:b0 + 2, :], in0=t1[:],
                                    in1=dis2[:, b0:b0 + 2, None]
                                    .to_broadcast([BW, 2, HID]),
                                    op=mybir.AluOpType.mult)
                        if use_cc:
                            nc.sync.dma_start(
                                out=hs2_bnc[:].rearrange("(j b) f -> j b f", j=BW),
                                in_=hs2_sb[:])
                            nc.gpsimd.collective_compute(
                                "AllGather", mybir.AluOpType.bypass,
                                replica_groups=[list(range(cfg.NCORES))],
                                ins=[hs2_bnc.opt()], outs=[hs2_pk.opt()],
                            )
                            nc.sync.dma_start(
                                out=hs2_tab[:cfg.HALF, 0:HID],
                                in_=hs2_pk[:cfg.HALF, :])
                            nc.sync.dma_start(
                                out=hs2_tab[cfg.HALF:, 0:HID],
                                in_=hs2_pk[cfg.HALF:, :])
                        else:
                            nc.sync.dma_start(
                                out=hs2_pk[:cfg.SHARD, :]
                                .rearrange("(j b) f -> j b f", j=BW),
                                in_=hs2_sb[:])
                            nc.sync.dma_start(out=hs2_tab[:, 0:HID],
                                              in_=hs2_pk[:])

                    # ============== PHASE C: layer-2 + projections ==============
                    if phases >= 3:
                        msg = gather_all(mpool, hs2_tab[:], "m")
                        S2 = {H: build_S(spool, H, f"s{H}") for H in HALVES}
                        # feature-major aggregation: aggT[f, node]
                        a2T_sb = usb.tile([HID, NBK * BW], BF16, tag="usb",
                                          name="a2T_sb")
                        for b0 in range(0, NBK, 2):
                            ps = pagg.tile([HID, 2, BW], F32, space="PSUM",
                                           tag="agg", name="ps2")
                            for k in (0, 1):
                                ent = entries[b0 + k]
                                for i, (H, t) in enumerate(ent):
                                    nc.tensor.matmul(
                                        out=ps[:, k, :], lhsT=msg[H](t),
                                        rhs=S2[H](t), start=(i == 0),
                                        stop=(i == len(ent) - 1))
                            nc.scalar.copy(
                                out=a2T_sb[:, b0 * BW:(b0 + 2) * BW],
                                in_=ps[:])
                        if not proj:
                            nc.sync.dma_start(
                                out=z_out.ap()[0:HID, :, :],
                                in_=a2T_sb[:].rearrange(
                                    "p (t q) -> p t q", q=2 * HID))
                        # zcatT = [Wmu | Wlv].T @ aggT  -> [2H, nodes]
                        zT_sb = usb.tile([2 * HID, NBK * BW], BF16, tag="usb",
                                         name="zT_sb")
                        CHK = 512
                        for n0 in (range(0, NBK * BW, CHK) if proj else []):
                            cn = min(CHK, NBK * BW - n0)
                            zT_ps = pproj.tile([2 * HID, CHK], F32,
                                               space="PSUM", tag="zT",
                                               name="zT_ps")
                            nc.tensor.matmul(out=zT_ps[:, :cn],
                                             lhsT=wml_sb[:],
                                             rhs=a2T_sb[:, n0:n0 + cn],
                                             start=True, stop=True)
                            nc.scalar.copy(out=zT_sb[:, n0:n0 + cn],
                                           in_=zT_ps[:, :cn])
                        # transpose back per 128-node pair, scale by dis
                        zcat_sb = usb.tile([P, NBK // 2, 2 * HID], BF16,
                                           tag="usb", name="zcat_sb")
                        for t in (range(NBK // 2) if proj else []):
                            z_ps = pz.tile([P, P], BF16, space="PSUM",
                                           tag="z", name="z_ps")
                            nc.tensor.transpose(
                                out=z_ps[:],
                                in_=zT_sb[:, t * P:(t + 1) * P],
                                identity=ident_bf[:])
                            nc.vector.tensor_tensor(
                                out=zcat_sb[:, t, :], in0=z_ps[:],
                                in1=disP[:, t, None].to_broadcast([P, P]),
                                op=mybir.AluOpType.mult)
                        if has_bias and proj:
                            nc.vector.tensor_tensor(
                                out=zcat_sb[:], in0=zcat_sb[:],
                                in1=bias_bc[:, None, HID:3 * HID]
                                .to_broadcast([P, NBK // 2, 2 * HID]),
                                op=mybir.AluOpType.add)
                        if proj:
                            nc.sync.dma_start(out=z_out.ap()[:],
                                              in_=zcat_sb[:])
                    if phases < 3:
                        zfill = usb.tile([P, NBK // 2, 2 * HID], BF16,
                                         tag="usb", name="zfill")
                        nc.vector.memset(zfill[:], 0)
                        nc.sync.dma_start(out=z_out.ap()[:], in_=zfill[:])

    nc.compile()
    return nc


_CACHE = {}


def _get_program(edge_index, cfg, has_bias):
    layout = build_layout(edge_index, cfg)
    key = (layout["ntA"], layout["ntB"], has_bias)
    if key not in _CACHE:
        _CACHE[key] = build_program(layout, cfg, has_bias)
    return _CACHE[key], layout


def make_in_maps(x, edge_index, W1, b1, Wmu, bmu, Wlv, blv, layout,
                 cfg=DEFAULT, has_bias=False):
    x = np.asarray(x, np.float32)
    xpad = np.zeros((cfg.NPAD, cfg.IN), np.float32)
    xpad[:x.shape[0]] = x
    xT = np.ascontiguousarray(xpad.T)
    wml = np.concatenate([np.asarray(Wmu, np.float32),
                          np.asarray(Wlv, np.float32)], axis=1)
    w1 = np.asarray(W1, np.float32)
    nTA, nTB = layout["nTA"], layout["nTB"]
    OFF = pack_offsets(cfg, nTA, nTB)

    def put_f32(pk, name, arr):
        arr = np.asarray(arr, np.float32)
        o = OFF[name]
        pk[:arr.shape[0], o:o + 2 * arr.shape[1]] = arr.view(np.int16)

    def put_bf16(pk, name, arr):
        import ml_dtypes
        arr = np.asarray(arr, np.float32).astype(ml_dtypes.bfloat16)
        o = OFF[name]
        pk[:arr.shape[0], o:o + arr.shape[1]] = arr.view(np.int16)

    maps = []
    for c in range(cfg.NCORES):
        pk = np.zeros((cfg.P, OFF["W"]), np.int16)
        cd = layout["cores"][c]
        put_bf16(pk, "xT", xT)
        put_f32(pk, "disG", layout["disG"])
        put_f32(pk, "w1", w1)
        put_f32(pk, "wml", wml)
        put_f32(pk, "dis", cd["dis"])
        put_f32(pk, "disP", cd["disP"])
        put_f32(pk, "dlA", cd["dlA"])
        put_f32(pk, "dlB", cd["dlB"])
        pk[:, OFF["idxA"]:OFF["idxA"] + 8 * nTA] = cd["idxA"]
        pk[:, OFF["idxB"]:OFF["idxB"] + 8 * nTB] = cd["idxB"]
        m = dict(pk=pk)
        if has_bias:
            m.update(b1=np.asarray(b1, np.float32).reshape(1, -1),
                     bmu=np.asarray(bmu, np.float32).reshape(1, -1),
                     blv=np.asarray(blv, np.float32).reshape(1, -1))
        maps.append(m)
    return maps


def unshard(results, cfg=DEFAULT):
    H = cfg.HID
    zmu_blocks, zlv_blocks = [], []
    for c in range(cfg.NCORES):
        z = np.asarray(results[c]["z"]).astype(np.float32)
        z4 = z.reshape(2, cfg.BW, cfg.NBK // 2, 2 * H)
        zjb = np.transpose(z4, (2, 0, 1, 3)).reshape(cfg.NBK, cfg.BW, 2 * H)
        # zjb[b, j, :]: node c*SHARD + b*BW + j
        zmu_blocks.append(zjb[:, :, 0:H].reshape(cfg.SHARD, H))
        zlv_blocks.append(zjb[:, :, H:2 * H].reshape(cfg.SHARD, H))
    return (np.concatenate(zmu_blocks, axis=0)[:cfg.N],
            np.concatenate(zlv_blocks, axis=0)[:cfg.N])


def kernel(x, edge_index, W1, b1, Wmu, bmu, Wlv, blv):
    cfg = DEFAULT
    has_bias = any(np.any(np.asarray(b)) for b in (b1, bmu, blv))
    nc, layout = _get_program(np.asarray(edge_index), cfg, has_bias)
    in_maps = make_in_maps(x, edge_index, W1, b1, Wmu, bmu, Wlv, blv,
                           layout, cfg, has_bias)
    res = run_bass_kernel_spmd(nc, in_maps, core_ids=list(range(cfg.NCORES)))
    return unshard(res.results, cfg)


# revision 48
# speedup vs baseline: 1.0201x; 1.0201x over previous
"""VGCN encoder (2-layer GCN, shared normalized adjacency) on 8 Trainium2 cores.

Strategy: node-partitioned graph parallelism. Nodes are padded to
NPAD = 8*6272 and core c owns nodes [6272c, 6272(c+1)), split into 98 buckets
of 64. All edges (plus one self-edge per node, which realizes both GCN's +1
degree and the self-loop term) are routed to the core owning their dst node,
bucketed by dst bucket, and aggregated on-device with one-hot matmuls over
128-edge tiles (bf16 operands, fp32 psum):

    agg[bucket] += S.T @ us[src]   (S[e, j] = dst_local[e] == j, built on DVE)

Messages are fetched with SWDGE dma_gather (4 queues round-robin) from a
bf16 DRAM table whose 256-byte rows ([HID bf16 payload | HID pad], the
minimum SWDGE element) are stored in a (core, partition, bucket)-major
permutation so bulk table writes are contiguous DMAs; the host permutes
gather indices to match. dis = 1/sqrt(deg) is precomputed on the host (pure
graph structure, like the edge bucketing itself).

Layer 1 input (x @ W1, x shipped in bf16) is computed REPLICATED on every
core for the whole graph - it is ~50 MFLOP, far cheaper than an AllGather
round - so the only collective is the AllGather of the layer-1 activations
(packed bf16, Shared-scratchpad output, expanded to padded rows locally).
Layer-2 aggregation runs feature-major (lhsT=msg) so both output projections
become 13 wide matmuls against [Wmu | Wlv] with one transpose per 128 nodes.

All host-prepared operands travel in ONE packed int16 input tensor (f32/bf16
sections bitcast on device) and both outputs leave in ONE tensor; together
with pipelined dispatch this keeps the per-execution dispatch overhead of the
axon tunnel (~0.8 ms flat per round trip) mostly off the measured
steady-state time.
"""

import sys

sys.path.insert(0, "/opt/trn_rl_repo")

import numpy as np

from concourse import bacc, mybir, tile
from concourse.bass_utils import run_bass_kernel_spmd
from concourse.masks import make_identity

F32 = mybir.dt.float32
BF16 = mybir.dt.bfloat16
I16 = mybir.dt.int16
I32 = mybir.dt.int32


class Cfg:
    def __init__(self, n=50000, e=800000, in_dim=128, hid=64, ncores=8,
                 shard_tiles=49, bw=64, half=32768, chunk_tiles=38, sbatch=16):
        self.N, self.E, self.IN, self.HID = n, e, in_dim, hid
        self.NCORES = ncores
        self.P = 128
        self.SHARD = shard_tiles * 128    # nodes per core
        self.NPAD = ncores * self.SHARD
        self.BW = bw                      # bucket width (psum partition dim)
        self.NBK = self.SHARD // bw       # buckets per core
        self.GBK = ncores * self.NBK      # global buckets
        self.HALF = half                  # gather-table split so int16 idx fit
        self.CH = chunk_tiles             # tiles (128 rows) per dma_gather
        self.SB = sbatch                  # tiles per batched one-hot build
        assert self.NPAD >= n and half <= 32768 and self.SHARD % bw == 0
        assert self.NBK <= 128 and self.NBK % 2 == 0


DEFAULT = Cfg()


def pack_offsets(cfg, nTA, nTB):
    """Column offsets (int16 units) of each section in the packed input."""
    off, out = 0, {}
    for name, width in (("xT", cfg.NPAD), ("w1", 2 * cfg.HID),
                        ("wml", 4 * cfg.HID), ("dis", 2 * cfg.NBK),
                        ("disG", 2 * cfg.NCORES * cfg.NBK),
                        ("disP", cfg.NBK), ("dlA", 2 * nTA),
                        ("dlB", 2 * nTB), ("idxA", 8 * nTA),
                        ("idxB", 8 * nTB)):
        out[name] = off
        off += width
    out["W"] = off
    return out


def build_layout(edge_index, cfg=DEFAULT):
    """Static per-core edge streams plus the (identical-across-cores) tile
    structure. Table row of node n: c*SHARD + (r%BW)*NBK + r//BW, r=n%SHARD."""
    src = np.asarray(edge_index[0], np.int64)
    dst = np.asarray(edge_index[1], np.int64)
    NBK, BW = cfg.NBK, cfg.BW

    deg = np.bincount(dst, minlength=cfg.NPAD).astype(np.float64) + 1.0
    dis = (1.0 / np.sqrt(deg)).astype(np.float32)   # padding nodes: dis=1

    per_core = []
    cnts = np.zeros((cfg.NCORES, NBK * 2), np.int64)
    for c in range(cfg.NCORES):
        m = (dst >= c * cfg.SHARD) & (dst < (c + 1) * cfg.SHARD)
        s = src[m]
        d = dst[m]
        selfn = np.arange(c * cfg.SHARD, (c + 1) * cfg.SHARD, dtype=np.int64)
        s = np.concatenate([s, selfn])
        d = np.concatenate([d, selfn])
        cc, rr = s // cfg.SHARD, s % cfg.SHARD
        row = cc * cfg.SHARD + (rr % BW) * NBK + rr // BW
        dr = d - c * cfg.SHARD
        b = dr // BW
        dl = dr % BW
        h = (row >= cfg.HALF).astype(np.int64)
        key = b * 2 + h
        order = np.argsort(key, kind="stable")
        row, dl, key = row[order], dl[order], key[order]
        per_core.append((row, dl, key))
        cnts[c] = np.bincount(key, minlength=NBK * 2)

    ntile = np.ceil(cnts.max(axis=0) / 128.0).astype(np.int64)
    ntA, ntB = ntile[0::2], ntile[1::2]
    nTA, nTB = int(ntA.sum()), int(ntB.sum())

    tbA = np.repeat(np.arange(NBK), ntA)
    tbB = np.repeat(np.arange(NBK), ntB)
    offA = np.concatenate([[0], np.cumsum(ntA)]) * 128
    offB = np.concatenate([[0], np.cumsum(ntB)]) * 128

    cores = []
    for c in range(cfg.NCORES):
        row, dl, key = per_core[c]
        bounds = np.searchsorted(key, np.arange(NBK * 2 + 1))
        idxA = np.zeros(nTA * 128, np.int64)
        dlA = np.full(nTA * 128, BW, np.int64)
        idxB = np.zeros(nTB * 128, np.int64)
        dlB = np.full(nTB * 128, BW, np.int64)
        for b in range(NBK):
            lo, hi = bounds[2 * b], bounds[2 * b + 1]
            o = offA[b]
            idxA[o:o + hi - lo] = row[lo:hi]
            dlA[o:o + hi - lo] = dl[lo:hi]
            lo, hi = bounds[2 * b + 1], bounds[2 * b + 2]
            o = offB[b]
            idxB[o:o + hi - lo] = row[lo:hi] - cfg.HALF
            dlB[o:o + hi - lo] = dl[lo:hi]

        def wrap(stream):
            a = stream.reshape(-1, 16).T.astype(np.int16)
            return np.tile(a, (8, 1))   # replicated across the 8 q7 cores

        # dis for own nodes in (partition=dl, bucket) layout
        dis_own = np.ascontiguousarray(
            dis[c * cfg.SHARD:(c + 1) * cfg.SHARD].reshape(NBK, BW).T)

        disP = np.ascontiguousarray(
            dis_own.reshape(BW, NBK // 2, 2).transpose(2, 0, 1)
            .reshape(2 * BW, NBK // 2))

        cores.append(dict(
            idxA=wrap(idxA), idxB=wrap(idxB),
            dlA=np.ascontiguousarray(dlA.reshape(-1, 128).T.astype(np.float32)),
            dlB=np.ascontiguousarray(dlB.reshape(-1, 128).T.astype(np.float32)),
            dis=dis_own, disP=disP,
        ))

    disG = np.concatenate([c["dis"] for c in cores], axis=1)
    return dict(ntA=tuple(int(x) for x in ntA), ntB=tuple(int(x) for x in ntB),
                tbA=tbA, tbB=tbB, nTA=nTA, nTB=nTB, cores=cores, disG=disG)


def build_program(layout, cfg=DEFAULT, has_bias=False, reps=1,
                  skip_cc=False, phases=3, gather_mode="gather", proj=True):
    """Emit the SPMD bass program (identical on all cores)."""
    nc = bacc.Bacc("TRN2", target_bir_lowering=False, debug=False,
                   num_devices=cfg.NCORES, num_swdge_queues=4)
    P, BW, NBK, HID = cfg.P, cfg.BW, cfg.NBK, cfg.HID
    nTA, nTB = layout["nTA"], layout["nTB"]
    tb = {0: layout["tbA"], 1: layout["tbB"]}
    nT = {0: nTA, 1: nTB}
    HALVES = [H for H in (0, 1) if nT[H] > 0]
    use_cc = cfg.NCORES > 1 and not skip_cc

    # ---------------- I/O (single packed input, single output) ----------
    OFF = pack_offsets(cfg, nTA, nTB)
    pk_in = nc.dram_tensor("pk", [P, OFF["W"]], I16, kind="ExternalInput")

    def fsec(name, n, rows=P):
        o = OFF[name]
        return pk_in.ap()[:rows, o:o + 2 * n].bitcast(F32)

    def isec(name, n):
        o = OFF[name]
        return pk_in.ap()[:, o:o + n]

    xT_b = pk_in.ap()[:, OFF["xT"]:OFF["xT"] + cfg.NPAD].bitcast(BF16)
    if has_bias:
        b1_in = nc.dram_tensor("b1", [1, HID], F32, kind="ExternalInput")
        bmu_in = nc.dram_tensor("bmu", [1, HID], F32, kind="ExternalInput")
        blv_in = nc.dram_tensor("blv", [1, HID], F32, kind="ExternalInput")
    z_out = nc.dram_tensor("z", [P, NBK // 2, 2 * HID], BF16,
                           kind="ExternalOutput")

    with tile.TileContext(nc) as tc:
        import contextlib
        stack = contextlib.ExitStack()
        with stack:
            dram = stack.enter_context(tc.tile_pool(name="dram", bufs=1, space="DRAM"))
            cpool = stack.enter_context(tc.tile_pool(name="const", bufs=1))

            us_tab = dram.tile([cfg.NPAD, 2 * HID], BF16)
            hs2_bnc = dram.tile([cfg.SHARD, HID], BF16)
            hs2_pk = dram.tile([cfg.NPAD, HID], BF16, addr_space="Shared")
            hs2_tab = dram.tile([cfg.NPAD, 2 * HID], BF16)

            w1_f = cpool.tile([cfg.IN, HID], F32)
            nc.sync.dma_start(out=w1_f[:], in_=fsec("w1", HID))
            w1_sb = cpool.tile([cfg.IN, HID], BF16)
            nc.vector.tensor_copy(out=w1_sb[:], in_=w1_f[:])
            wml_f = cpool.tile([HID, 2 * HID], F32)
            nc.sync.dma_start(out=wml_f[:], in_=fsec("wml", 2 * HID, HID))
            wml_sb = cpool.tile([HID, 2 * HID], BF16)
            nc.vector.tensor_copy(out=wml_sb[:], in_=wml_f[:])
            dis_own = cpool.tile([BW, NBK], F32)
            nc.sync.dma_start(out=dis_own[:], in_=fsec("dis", NBK, BW))
            disP = cpool.tile([P, NBK // 2], F32)
            nc.sync.dma_start(out=disP[:], in_=fsec("disP", NBK // 2))
            dis2 = cpool.tile([BW, NBK], F32)
            nc.vector.tensor_tensor(out=dis2[:], in0=dis_own[:],
                                    in1=dis_own[:], op=mybir.AluOpType.mult)
            dis_g = cpool.tile([BW, cfg.NCORES * NBK], F32)
            nc.sync.dma_start(out=dis_g[:],
                              in_=fsec("disG", cfg.NCORES * NBK, BW))

            iota_i = cpool.tile([P, BW], I32)
            nc.gpsimd.iota(iota_i[:], pattern=[[1, BW]], base=0,
                           channel_multiplier=0)
            iota_b = cpool.tile([P, BW], BF16)
            nc.vector.tensor_copy(out=iota_b[:], in_=iota_i[:])

            ident = cpool.tile([P, P], F32)
            make_identity(nc, ident[:])
            ident_bf = cpool.tile([P, P], BF16)
            nc.vector.tensor_copy(out=ident_bf[:], in_=ident[:])

            idx_sb, dl_sb = {}, {}
            for H in HALVES:
                idx_sb[H] = cpool.tile([P, nT[H] * 8], I16, tag=f"idx{H}",
                                       name=f"idx{H}")
                nc.sync.dma_start(out=idx_sb[H][:],
                                  in_=isec("idxA" if H == 0 else "idxB",
                                           nT[H] * 8))
                dl_f = cpool.tile([P, nT[H]], F32, tag=f"dlf{H}",
                                  name=f"dlf{H}")
                nc.sync.dma_start(out=dl_f[:],
                                  in_=fsec("dlA" if H == 0 else "dlB",
                                           nT[H]))
                dl_sb[H] = cpool.tile([P, nT[H]], BF16, tag=f"dl{H}",
                                      name=f"dls{H}")
                nc.vector.tensor_copy(out=dl_sb[H][:], in_=dl_f[:])

            if has_bias:
                brow = cpool.tile([1, 3 * HID], F32)
                nc.sync.dma_start(out=brow[:, 0:HID], in_=b1_in.ap()[:])
                nc.sync.dma_start(out=brow[:, HID:2 * HID], in_=bmu_in.ap()[:])
                nc.sync.dma_start(out=brow[:, 2 * HID:], in_=blv_in.ap()[:])
                bias_bc = cpool.tile([P, 3 * HID], F32)
                nc.gpsimd.partition_broadcast(bias_bc[:], brow[:])

            def build_S(spool, H, tag):
                tiles = []
                for t0 in range(0, nT[H], cfg.SB):
                    tn = min(cfg.SB, nT[H] - t0)
                    st = spool.tile([P, cfg.SB, BW], BF16, tag=tag,
                                    name=f"S{tag}")
                    nc.vector.tensor_tensor(
                        out=st[:, :tn, :],
                        in0=dl_sb[H][:, t0:t0 + tn].to_broadcast([P, tn, BW]),
                        in1=iota_b[:, None, :].to_broadcast([P, tn, BW]),
                        op=mybir.AluOpType.is_equal,
                    )
                    tiles.append(st)

                def one(t):
                    return tiles[t // cfg.SB][:, t % cfg.SB, :]

                return one

            qctr = [0]

            def gather_all(mpool, table, tagp):
                """Gather both halves' chunks, interleaved so the earliest
                tiles of BOTH halves land first (bucket 0 needs both)."""
                tiles = {H: [] for H in HALVES}
                starts = {H: list(range(0, nT[H], cfg.CH)) for H in HALVES}
                order = []
                for i in range(max(len(starts[H]) for H in HALVES)):
                    for H in HALVES:
                        if i < len(starts[H]):
                            order.append((H, starts[H][i]))
                for H, t0 in order:
                    tn = min(cfg.CH, nT[H] - t0)
                    mt = mpool.tile([P, cfg.CH, 2 * HID], BF16,
                                    tag=f"{tagp}{H}", name=f"M{tagp}{H}")
                    if gather_mode == "copy":
                        nc.sync.dma_start(
                            out=mt[:, :tn, :],
                            in_=table[:tn * 128, :]
                            .rearrange("(t p) f -> p t f", p=128))
                    else:
                        nc.gpsimd.dma_gather(
                            out_ap=mt[:, :tn, :],
                            in_ap=(table[:min(cfg.HALF, cfg.NPAD), :]
                                   if H == 0 else table[cfg.HALF:, :]),
                            idxs_ap=idx_sb[H][:, t0 * 8:(t0 + tn) * 8],
                            num_idxs=tn * 128, num_idxs_reg=tn * 128,
                            elem_size=2 * HID,
                            single_packet=(tn * 128 <= 512),
                            queue_num=qctr[0] % 4,
                        )
                        qctr[0] += 1
                    tiles[H].append(mt)

                def make(H):
                    tl = tiles[H]
                    return lambda t: tl[t // cfg.CH][:, t % cfg.CH, 0:HID]

                return {H: make(H) for H in HALVES}

            entries = [[] for _ in range(NBK)]
            for H in HALVES:
                for t, b in enumerate(tb[H]):
                    entries[int(b)].append((H, t))

            for _rep in range(reps):
                # ========= PHASE A: u = x @ W1, scaled by dis -> us table ====
                it_stack = contextlib.ExitStack()
                with it_stack:
                    xa = it_stack.enter_context(tc.tile_pool(name="xa", bufs=2))
                    pu = it_stack.enter_context(
                        tc.tile_pool(name="pu", bufs=1, space="PSUM"))
                    usb = it_stack.enter_context(tc.tile_pool(name="usb", bufs=2))
                    spool = it_stack.enter_context(tc.tile_pool(name="spool", bufs=2))
                    mpool = it_stack.enter_context(tc.tile_pool(name="mpool", bufs=6))
                    pagg = it_stack.enter_context(
                        tc.tile_pool(name="pagg", bufs=3, space="PSUM"))
                    hb = it_stack.enter_context(tc.tile_pool(name="hb", bufs=2))
                    small = it_stack.enter_context(tc.tile_pool(name="small", bufs=3))
                    ptr = it_stack.enter_context(
                        tc.tile_pool(name="ptr", bufs=1, space="PSUM"))
                    pproj = it_stack.enter_context(
                        tc.tile_pool(name="pproj", bufs=2, space="PSUM"))
                    pz = it_stack.enter_context(
                        tc.tile_pool(name="pz", bufs=1, space="PSUM"))

                    XC = 16  # buckets per xT DMA / psum bank / scale batch
                    if phases < 1:
                        zfill0 = usb.tile([BW, NBK, HID], BF16, tag="usbb",
                                          name="zfill0")
                        nc.vector.memset(zfill0[:], 0)
                    c2_order = [5, 6, 7, 0, 1, 2, 3, 4][:cfg.NCORES]
                    for c2 in (c2_order if phases >= 1 else []):
                        us_blk = usb.tile([BW, NBK, HID], BF16, tag="usbb",
                                          name="us_blk")
                        for B0 in range(0, NBK, XC):
                            bn = min(XC, NBK - B0)
                            xtb = xa.tile([P, XC, BW], BF16, tag="xtb",
                                          name="xtb")
                            nc.sync.dma_start(
                                out=xtb[:, :bn, :],
                                in_=xT_b[:, c2 * cfg.SHARD + B0 * BW:
                                         c2 * cfg.SHARD + (B0 + bn) * BW]
                                .rearrange("p (t q) -> p t q", t=bn))
                            ups = pu.tile([BW, XC, HID], F32, space="PSUM",
                                          tag="u", name="ups")
                            for j in range(bn):
                                nc.tensor.matmul(out=ups[:, j, :],
                                                 lhsT=xtb[:, j, :],
                                                 rhs=w1_sb[:],
                                                 start=True, stop=True)
                            nc.vector.tensor_tensor(
                                out=us_blk[:, B0:B0 + bn, :],
                                in0=ups[:, :bn, :],
                                in1=dis_g[:, c2 * NBK + B0:
                                          c2 * NBK + B0 + bn, None]
                                .to_broadcast([BW, bn, HID]),
                                op=mybir.AluOpType.mult)
                        nc.sync.dma_start(
                            out=us_tab[c2 * cfg.SHARD:(c2 + 1) * cfg.SHARD,
                                       0:HID]
                            .rearrange("(j b) f -> j b f", j=BW),
                            in_=us_blk[:])

                    # ================= PHASE B: layer-1 aggregation =============
                    if phases >= 2:
                        msg = gather_all(mpool, us_tab[:], "m")
                        S1 = {H: build_S(spool, H, f"s{H}") for H in HALVES}
                        hs2_sb = usb.tile([BW, NBK, HID], BF16, tag="usbb",
                                          name="hs2_sb")
                        for b0 in range(0, NBK, 2):
                            ps = pagg.tile([BW, 2, HID], F32, space="PSUM",
                                           tag="agg", name="ps1")
                            for k in (0, 1):
                                ent = entries[b0 + k]
                                for i, (H, t) in enumerate(ent):
                                    nc.tensor.matmul(
                                        out=ps[:, k, :], lhsT=S1[H](t),
                                        rhs=msg[H](t), start=(i == 0),
                                        stop=(i == len(ent) - 1))
                            t1 = hb.tile([BW, 2, HID], F32, tag="h",
                                         name="t1")
                            if has_bias:
                                # h1 = relu(ps*dis + b); hs2 = h1*dis
                                dpair = dis_own[:, b0:b0 + 2, None] \
                                    .to_broadcast([BW, 2, HID])
                                nc.vector.tensor_tensor(
                                    out=t1[:], in0=ps[:], in1=dpair,
                                    op=mybir.AluOpType.mult)
                                nc.vector.tensor_tensor(
                                    out=t1[:], in0=t1[:],
                                    in1=bias_bc[:BW, None, 0:HID]
                                    .to_broadcast([BW, 2, HID]),
                                    op=mybir.AluOpType.add)
                                nc.vector.tensor_relu(out=t1[:], in_=t1[:])
                                nc.vector.tensor_tensor(
                                    out=hs2_sb[:, b0:b0 + 2, :], in0=t1[:],
                                    in1=dpair, op=mybir.AluOpType.mult)
                            else:
                                # dis>0: relu(ps*dis)*dis == relu(ps)*dis^2
                                nc.vector.tensor_relu(out=t1[:], in_=ps[:])
                                nc.vector.tensor_tensor(
                                    out=hs2_sb[:, b0:b0 + 2, :], in0=t1[:],
                                    in1=dis2[:, b0:b0 + 2, None]
                                    .to_broadcast([BW, 2, HID]),
                                    op=mybir.AluOpType.mult)
                        if use_cc:
                            nc.sync.dma_start(
                                out=hs2_bnc[:].rearrange("(j b) f -> j b f", j=BW),
                                in_=hs2_sb[:])
                            nc.gpsimd.collective_compute(
                                "AllGather", mybir.AluOpType.bypass,
                                replica_groups=[list(range(cfg.NCORES))],
                                ins=[hs2_bnc.opt()], outs=[hs2_pk.opt()],
                            )
                            nc.sync.dma_start(out=hs2_tab[:, 0:HID],
                                              in_=hs2_pk[:])
                        else:
                            nc.sync.dma_start(
                                out=hs2_pk[:cfg.SHARD, :]
                                .rearrange("(j b) f -> j b f", j=BW),
                                in_=hs2_sb[:])
                            nc.sync.dma_start(out=hs2_tab[:, 0:HID],
                                              in_=hs2_pk[:])

                    # ============== PHASE C: layer-2 + projections ==============
                    if phases >= 3:
                        msg = gather_all(mpool, hs2_tab[:], "m")
                        S2 = {H: build_S(spool, H, f"s{H}") for H in HALVES}
                        # feature-major aggregation: aggT[f, node]
                        a2T_sb = usb.tile([HID, NBK * BW], BF16, tag="usb",
                                          name="a2T_sb")
                        for b0 in range(0, NBK, 2):
                            ps = pagg.tile([HID, 2, BW], F32, space="PSUM",
                                           tag="agg", name="ps2")
                            for k in (0, 1):
                                ent = entries[b0 + k]
                                for i, (H, t) in enumerate(ent):
                                    nc.tensor.matmul(
                                        out=ps[:, k, :], lhsT=msg[H](t),
                                        rhs=S2[H](t), start=(i == 0),
                                        stop=(i == len(ent) - 1))
                            nc.scalar.copy(
                                out=a2T_sb[:, b0 * BW:(b0 + 2) * BW],
                                in_=ps[:])
                        if not proj:
                            nc.sync.dma_start(
                                out=z_out.ap()[0:HID, :, :],
                                in_=a2T_sb[:].rearrange(
                                    "p (t q) -> p t q", q=2 * HID))
                        # zcatT = [Wmu | Wlv].T @ aggT  -> [2H, nodes]
                        zT_sb = usb.tile([2 * HID, NBK * BW], BF16, tag="usb",
                                         name="zT_sb")
                        CHK = 512
                        for n0 in (range(0, NBK * BW, CHK) if proj else []):
                            cn = min(CHK, NBK * BW - n0)
                            zT_ps = pproj.tile([2 * HID, CHK], F32,
                                               space="PSUM", tag="zT",
                                               name="zT_ps")
                            nc.tensor.matmul(out=zT_ps[:, :cn],
                                             lhsT=wml_sb[:],
                                             rhs=a2T_sb[:, n0:n0 + cn],
                                             start=True, stop=True)
                            nc.scalar.copy(out=zT_sb[:, n0:n0 + cn],
                                           in_=zT_ps[:, :cn])
                        # transpose back per 128-node pair, scale by dis
                        zcat_sb = usb.tile([P, NBK // 2, 2 * HID], BF16,
                                           tag="usb", name="zcat_sb")
                        for t in (range(NBK // 2) if proj else []):
                            z_ps = pz.tile([P, P], BF16, space="PSUM",
                                           tag="z", name="z_ps")
                            nc.tensor.transpose(
                                out=z_ps[:],
                                in_=zT_sb[:, t * P:(t + 1) * P],
                                identity=ident_bf[:])
                            nc.vector.tensor_tensor(
                                out=zcat_sb[:, t, :], in0=z_ps[:],
                                in1=disP[:, t, None].to_broadcast([P, P]),
                                op=mybir.AluOpType.mult)
                        if has_bias and proj:
                            nc.vector.tensor_tensor(
                                out=zcat_sb[:], in0=zcat_sb[:],
                                in1=bias_bc[:, None, HID:3 * HID]
                                .to_broadcast([P, NBK // 2, 2 * HID]),
                                op=mybir.AluOpType.add)
                        if proj:
                            nc.sync.dma_start(out=z_out.ap()[:],
                                              in_=zcat_sb[:])
                    if phases < 3:
                        zfill = usb.tile([P, NBK // 2, 2 * HID], BF16,
                                         tag="usb", name="zfill")
                        nc.vector.memset(zfill[:], 0)
                        nc.sync.dma_start(out=z_out.ap()[:], in_=zfill[:])

    nc.compile()
    return nc


_CACHE = {}


def _get_program(edge_index, cfg, has_bias):
    layout = build_layout(edge_index, cfg)
    key = (layout["ntA"], layout["ntB"], has_bias)
    if key not in _CACHE:
        _CACHE[key] = build_program(layout, cfg, has_bias)
    return _CACHE[key], layout


def make_in_maps(x, edge_index, W1, b1, Wmu, bmu, Wlv, blv, layout,
                 cfg=DEFAULT, has_bias=False):
    x = np.asarray(x, np.float32)
    xpad = np.zeros((cfg.NPAD, cfg.IN), np.float32)
    xpad[:x.shape[0]] = x
    xT = np.ascontiguousarray(xpad.T)
    wml = np.concatenate([np.asarray(Wmu, np.float32),
                          np.asarray(Wlv, np.float32)], axis=1)
    w1 = np.asarray(W1, np.float32)
    nTA, nTB = layout["nTA"], layout["nTB"]
    OFF = pack_offsets(cfg, nTA, nTB)

    def put_f32(pk, name, arr):
        arr = np.asarray(arr, np.float32)
        o = OFF[name]
        pk[:arr.shape[0], o:o + 2 * arr.shape[1]] = arr.view(np.int16)

    def put_bf16(pk, name, arr):
        import ml_dtypes
        arr = np.asarray(arr, np.float32).astype(ml_dtypes.bfloat16)
        o = OFF[name]
        pk[:arr.shape[0], o:o + arr.shape[1]] = arr.view(np.int16)

    maps = []
    for c in range(cfg.NCORES):
        pk = np.zeros((cfg.P, OFF["W"]), np.int16)
        cd = layout["cores"][c]
        put_bf16(pk, "xT", xT)
        put_f32(pk, "disG", layout["disG"])
        put_f32(pk, "w1", w1)
        put_f32(pk, "wml", wml)
        put_f32(pk, "dis", cd["dis"])
        put_f32(pk, "disP", cd["disP"])
        put_f32(pk, "dlA", cd["dlA"])
        put_f32(pk, "dlB", cd["dlB"])
        pk[:, OFF["idxA"]:OFF["idxA"] + 8 * nTA] = cd["idxA"]
        pk[:, OFF["idxB"]:OFF["idxB"] + 8 * nTB] = cd["idxB"]
        m = dict(pk=pk)
        if has_bias:
            m.update(b1=np.asarray(b1, np.float32).reshape(1, -1),
                     bmu=np.asarray(bmu, np.float32).reshape(1, -1),
                     blv=np.asarray(blv, np.float32).reshape(1, -1))
        maps.append(m)
    return maps


def unshard(results, cfg=DEFAULT):
    H = cfg.HID
    zmu_blocks, zlv_blocks = [], []
    for c in range(cfg.NCORES):
        z = np.asarray(results[c]["z"]).astype(np.float32)
        z4 = z.reshape(2, cfg.BW, cfg.NBK // 2, 2 * H)
        zjb = np.transpose(z4, (2, 0, 1, 3)).reshape(cfg.NBK, cfg.BW, 2 * H)
        # zjb[b, j, :]: node c*SHARD + b*BW + j
        zmu_blocks.append(zjb[:, :, 0:H].reshape(cfg.SHARD, H))
        zlv_blocks.append(zjb[:, :, H:2 * H].reshape(cfg.SHARD, H))
    return (np.concatenate(zmu_blocks, axis=0)[:cfg.N],
            np.concatenate(zlv_blocks, axis=0)[:cfg.N])


def kernel(x, edge_index, W1, b1, Wmu, bmu, Wlv, blv):
    cfg = DEFAULT
    has_bias = any(np.any(np.asarray(b)) for b in (b1, bmu, blv))
    nc, layout = _get_program(np.asarray(edge_index), cfg, has_bias)
    in_maps = make_in_maps(x, edge_index, W1, b1, Wmu, bmu, Wlv, blv,
                           layout, cfg, has_bias)
    res = run_bass_kernel_spmd(nc, in_maps, core_ids=list(range(cfg.NCORES)))
    return unshard(res.results, cfg)
